# revision 26
# baseline (speedup 1.0000x reference)
"""LogicLayer Trainium2 kernel.

out[b, n] = sum_k softmax(w[n])_k * gate_k(a1, a2),  a1 = x[b, i1[n]], a2 = x[b, i2[n]]

All 16 differentiable gates are affine in {1, a1, a2, a1*a2}:
    out[b, n] = A0[n] + A1[n]*a1 + A2[n]*a2 + Ap[n]*a1*a2
with A* = softmax(w[n]) @ C for a constant [16, 4] table C. A* is tiny
([8192, 4] total) and is precomputed on the host, so the device kernel is a
pure gather + 2-DVE-ops-per-slot + write pipeline.

Device plan (8 NeuronCores, neuron-sharded: 1024 neurons x full 2048 batch each):
  - x is shipped transposed and cast to f16 (xt [8192, 2048]) so one neuron's
    input column is a contiguous 4KB row in HBM. The correctness gate is
    rel_err < 2e-2; f16 gathers + f16 output writes land at ~3e-4 while
    halving HBM traffic vs f32 (~12MB/core instead of ~24MB).
  - gathers: 16 single-offset gpsimd.indirect_dma_start calls (multi-offset
    offset-APs and >32KB dest offsets within one dest tile both break the
    real SWDGE descriptor generator, although CoreSim accepts them), each
    into its own small per-slot tile.
  - all working tiles are static f16 SBUF residents (~100KB/partition), so
    there is no pool-buffer recycling and no resulting Q7 stalls.
  - inner loop is 2 fused DVE ops per 128-neuron slot (f16 data hits the
    2x-packed DVE mode):
        t   = (Ap*g2 + A1) * g1          (affine_mul_reduce)
        out = (A2*g2 + A0) + t           (affine_then_add)
  - output is written neuron-major [1024, 2048] f16, one write per slot so
    writes overlap the remaining gathers; host reassembles/transposes/upcasts.
"""

import numpy as np

BATCH = 2048
NIN = 8192
NNEUR = 8192
NCORES = 8
NN = NNEUR // NCORES  # neurons per core (1024)
NB = BATCH            # full batch per core
SLOTS = NN // 128     # 8
ACT_SLOTS = 5         # slots < this offload their affines to the ACT engine;
                      # later slots use DVE tensor_scalar so the tail after the
                      # last gather avoids the cross-engine ACT chain


# gate -> (c0, c1, c2, cp) so gate_k(a1,a2) = c0 + c1*a1 + c2*a2 + cp*a1*a2
GATE_COEF = np.array(
    [
        [0, 0, 0, 0],    # FALSE
        [0, 0, 0, 1],    # AND
        [0, 1, 0, -1],   # a1 AND NOT a2
        [0, 1, 0, 0],    # a1
        [0, 0, 1, -1],   # NOT a1 AND a2
        [0, 0, 1, 0],    # a2
        [0, 1, 1, -2],   # XOR
        [0, 1, 1, -1],   # OR
        [1, -1, -1, 1],  # NOR
        [1, -1, -1, 2],  # XNOR
        [1, 0, -1, 0],   # NOT a2
        [1, 0, -1, 1],   # a1 OR NOT a2
        [1, -1, 0, 0],   # NOT a1
        [1, -1, 0, 1],   # NOT a1 OR a2
        [1, 0, 0, -1],   # NAND
        [1, 0, 0, 0],    # TRUE
    ],
    dtype=np.float32,
)  # [16, 4]

_CACHE = {}


def _build_nc():
    import concourse.bacc as bacc
    import concourse.bass as bass
    import concourse.mybir as mybir
    from concourse.tile import TileContext

    f32 = mybir.dt.float32
    f16 = mybir.dt.float16
    f8 = mybir.dt.float8e3
    i32 = mybir.dt.int32

    nc = bacc.Bacc("TRN2")
    xt = nc.dram_tensor("xt", [NIN, NB], f16, kind="ExternalInput")
    # fp8-e3m4 copy of xt: the ACT-slot g1 gathers read this (the ACT engine
    # upconverts on read), saving HBM bytes at ~1.8e-3 total rel err
    xt8 = nc.dram_tensor("xt8", [NIN, NB], f8, kind="ExternalInput")
    # io[p, 2*s+o] = row index of operand o for neuron (slot s, partition p)
    io = nc.dram_tensor("io", [128, SLOTS * 2], i32, kind="ExternalInput")
    # ac[p, c, s] = coefficient A_c for neuron (slot s, partition p)
    ac = nc.dram_tensor("ac", [128, 4, SLOTS], f32, kind="ExternalInput")
    yt = nc.dram_tensor("yt", [NN, NB], f16, kind="ExternalOutput")

    with TileContext(nc) as tc:
        with tc.tile_pool(name="all", bufs=1) as pool:
            it = pool.tile([128, SLOTS * 2], i32)
            nc.sync.dma_start(it[:], io[:])
            act = pool.tile([128, 4, SLOTS], f32)
            nc.sync.dma_start(act[:], ac[:])

            # one small tile per gather: the SWDGE descriptor generator
            # mangles dest offsets beyond ~32KB within a single dest AP/tile,
            # so each indirect call targets its own tile at offset ~0.
            # ACT-path slots read g1 in fp8; the rest in f16.
            g1t = [
                pool.tile([128, NB], f8 if s < ACT_SLOTS else f16,
                          name=f"g1_{s}")
                for s in range(SLOTS)
            ]
            g2t = [pool.tile([128, NB], f16, name=f"g2_{s}") for s in range(SLOTS)]
            uv = [pool.tile([128, 2, NB], f16, name=f"uv{s}") for s in range(SLOTS)]
            ot = pool.tile([128, SLOTS, NB], f16)

            # Hand-tuned gather order: the ~1.4us-spaced indirect calls define
            # each tile's arrival time, so order them to keep both consumer
            # engines fed just-in-time — slot 0's g1 first (starts the serial
            # ACT chain ASAP), then the TS slots' g1 (DVE affines), then each
            # engine's next input interleaved in consumption order.
            order = [(0, 0), (5, 0), (6, 0), (7, 0), (5, 1), (1, 0), (6, 1),
                     (2, 0), (7, 1), (3, 0), (0, 1), (4, 0), (1, 1), (2, 1),
                     (3, 1), (4, 1)]
            for s, o in order:
                dst = g1t[s] if o == 0 else g2t[s]
                src = xt8 if (o == 0 and s < ACT_SLOTS) else xt
                nc.gpsimd.indirect_dma_start(
                    out=dst[:], out_offset=None,
                    in_=src[:],
                    in_offset=bass.IndirectOffsetOnAxis(
                        ap=it[:, 2 * s + o:2 * s + o + 1], axis=0),
                )

            # out = (Ap*g1 + A2)*g2 + (A1*g1 + A0):
            #   u = Ap*g1 + A2 ; v = A1*g1 + A0 ; out = u*g2 + v
            # Affines read only g1 (gathered first). Slots < ACT_SLOTS run
            # them on the scalar (ACT) engine; later slots run them as
            # 4x-packed DVE tensor_scalar ops, emitted FIRST so the in-order
            # DVE stream executes them during the g1-only phase, before the
            # g2-paced tensor_tensor chain.
            def slot_aps(s):
                return (g1t[s][:], g2t[s][:],
                        uv[s][:, 0, :], uv[s][:, 1, :],
                        act[:, 0, s:s + 1], act[:, 1, s:s + 1],
                        act[:, 2, s:s + 1], act[:, 3, s:s + 1])

            for s in range(ACT_SLOTS, SLOTS):
                g1, g2, u, v, A0, A1, A2, Ap = slot_aps(s)
                nc.vector.tensor_scalar(u, g1, Ap, A2,
                                        mybir.AluOpType.mult,
                                        mybir.AluOpType.add)
                nc.vector.tensor_scalar(v, g1, A1, A0,
                                        mybir.AluOpType.mult,
                                        mybir.AluOpType.add)

            # ACT engine stream: the serial chain u0,v0..u4,v4 paces the
            # ACT-path slots
            for s in range(ACT_SLOTS):
                g1, g2, u, v, A0, A1, A2, Ap = slot_aps(s)
                nc.scalar.activation(
                    u, g1, mybir.ActivationFunctionType.Identity,
                    bias=A2, scale=Ap)
                nc.scalar.activation(
                    v, g1, mybir.ActivationFunctionType.Identity,
                    bias=A0, scale=A1)

            # DVE tensor_tensor chain + writes: TS slots first (their u,v and
            # g2 are ready earliest), then the ACT slots in order
            tt_order = list(range(ACT_SLOTS, SLOTS)) + list(range(ACT_SLOTS))
            for s in tt_order:
                g1, g2, u, v, A0, A1, A2, Ap = slot_aps(s)
                nc.vector.tensor_mul(ot[:, s, :], u, g2)
                nc.vector.tensor_add(ot[:, s, :], ot[:, s, :], v)
                if s == tt_order[-1]:
                    # final write split across two HWDGE queues so its data
                    # and completion receipt land sooner
                    h = NB // 2
                    nc.sync.dma_start(yt[s * 128:(s + 1) * 128, 0:h],
                                      ot[:, s, 0:h])
                    nc.scalar.dma_start(yt[s * 128:(s + 1) * 128, h:NB],
                                        ot[:, s, h:NB])
                else:
                    nc.sync.dma_start(yt[s * 128:(s + 1) * 128, :], ot[:, s, :])

    nc.compile()
    return nc


def _prep_core_inputs(x, w, conn_indices):
    """Host-side shard/layout prep. Returns list of per-core input dicts."""
    import ml_dtypes

    xT = x.T
    xt = np.ascontiguousarray(xT.astype(np.float16))  # [NIN, BATCH] f16, shared
    xt8 = np.ascontiguousarray(
        xT.astype(ml_dtypes.float8_e3m4)).view(np.uint8)
    # A = softmax(w) @ GATE_COEF, [NNEUR, 4] — tiny; compute on host in f64
    ew = np.exp(w.astype(np.float64))
    probs = ew / ew.sum(axis=1, keepdims=True)
    A = (probs @ GATE_COEF.astype(np.float64)).astype(np.float32)
    maps = []
    for c in range(NCORES):
        n0 = c * NN
        # neuron n0 + s*128 + p -> partition p, slot s
        idx = conn_indices[n0:n0 + NN, :].reshape(SLOTS, 128, 2)
        io = idx.transpose(1, 0, 2).reshape(128, SLOTS * 2)
        ac = A[n0:n0 + NN, :].reshape(SLOTS, 128, 4).transpose(1, 2, 0)
        maps.append({
            "xt": xt,
            "xt8": xt8,
            "io": np.ascontiguousarray(io).astype(np.int32),
            "ac": np.ascontiguousarray(ac),
        })
    return maps


def run_cores(in_maps, trace=False):
    from concourse.bass_utils import run_bass_kernel_spmd

    if "nc" not in _CACHE:
        _CACHE["nc"] = _build_nc()
    return run_bass_kernel_spmd(
        _CACHE["nc"], in_maps, core_ids=list(range(NCORES)), trace=trace
    )


def _assemble(results):
    out = np.empty((BATCH, NNEUR), dtype=np.float32)
    for c in range(NCORES):
        n0 = c * NN
        out[:, n0:n0 + NN] = results[c]["yt"].T.astype(np.float32)
    return out


def kernel(x, w, conn_indices):
    x = np.asarray(x, dtype=np.float32)
    w = np.asarray(w, dtype=np.float32)
    conn_indices = np.asarray(conn_indices)
    in_maps = _prep_core_inputs(x, w, conn_indices)
    res = run_cores(in_maps)
    return _assemble([r for r in res.results])


# revision 28
# speedup vs baseline: 1.0552x; 1.0552x over previous
"""LogicLayer Trainium2 kernel.

out[b, n] = sum_k softmax(w[n])_k * gate_k(a1, a2),  a1 = x[b, i1[n]], a2 = x[b, i2[n]]

All 16 differentiable gates are affine in {1, a1, a2, a1*a2}:
    out[b, n] = A0[n] + A1[n]*a1 + A2[n]*a2 + Ap[n]*a1*a2
with A* = softmax(w[n]) @ C for a constant [16, 4] table C. A* is tiny
([8192, 4] total) and is precomputed on the host, so the device kernel is a
pure gather + 2-DVE-ops-per-slot + write pipeline.

Device plan (8 NeuronCores, neuron-sharded: 1024 neurons x full 2048 batch each):
  - x is shipped transposed and cast to f16 (xt [8192, 2048]) so one neuron's
    input column is a contiguous 4KB row in HBM. The correctness gate is
    rel_err < 2e-2; f16 gathers + f16 output writes land at ~3e-4 while
    halving HBM traffic vs f32 (~12MB/core instead of ~24MB).
  - gathers: 16 single-offset gpsimd.indirect_dma_start calls (multi-offset
    offset-APs and >32KB dest offsets within one dest tile both break the
    real SWDGE descriptor generator, although CoreSim accepts them), each
    into its own small per-slot tile.
  - all working tiles are static f16 SBUF residents (~100KB/partition), so
    there is no pool-buffer recycling and no resulting Q7 stalls.
  - inner loop is 2 fused DVE ops per 128-neuron slot (f16 data hits the
    2x-packed DVE mode):
        t   = (Ap*g2 + A1) * g1          (affine_mul_reduce)
        out = (A2*g2 + A0) + t           (affine_then_add)
  - output is written neuron-major [1024, 2048] f16, one write per slot so
    writes overlap the remaining gathers; host reassembles/transposes/upcasts.
"""

import numpy as np

BATCH = 2048
NIN = 8192
NNEUR = 8192
NCORES = 8
NN = NNEUR // NCORES  # neurons per core (1024)
NB = BATCH            # full batch per core
SLOTS = NN // 128     # 8
ACT_SLOTS = 4         # slots < this offload their affines to the ACT engine;
                      # later slots use DVE tensor_scalar so the tail after the
                      # last gather avoids the cross-engine ACT chain


# gate -> (c0, c1, c2, cp) so gate_k(a1,a2) = c0 + c1*a1 + c2*a2 + cp*a1*a2
GATE_COEF = np.array(
    [
        [0, 0, 0, 0],    # FALSE
        [0, 0, 0, 1],    # AND
        [0, 1, 0, -1],   # a1 AND NOT a2
        [0, 1, 0, 0],    # a1
        [0, 0, 1, -1],   # NOT a1 AND a2
        [0, 0, 1, 0],    # a2
        [0, 1, 1, -2],   # XOR
        [0, 1, 1, -1],   # OR
        [1, -1, -1, 1],  # NOR
        [1, -1, -1, 2],  # XNOR
        [1, 0, -1, 0],   # NOT a2
        [1, 0, -1, 1],   # a1 OR NOT a2
        [1, -1, 0, 0],   # NOT a1
        [1, -1, 0, 1],   # NOT a1 OR a2
        [1, 0, 0, -1],   # NAND
        [1, 0, 0, 0],    # TRUE
    ],
    dtype=np.float32,
)  # [16, 4]

_CACHE = {}


def _build_nc():
    import concourse.bacc as bacc
    import concourse.bass as bass
    import concourse.mybir as mybir
    from concourse.tile import TileContext

    f32 = mybir.dt.float32
    f16 = mybir.dt.float16
    f8 = mybir.dt.float8e3
    i32 = mybir.dt.int32

    nc = bacc.Bacc("TRN2")
    xt = nc.dram_tensor("xt", [NIN, NB], f16, kind="ExternalInput")
    # fp8-e3m4 copy of xt: the ACT-slot g1 gathers read this (the ACT engine
    # upconverts on read), saving HBM bytes at ~1.8e-3 total rel err
    xt8 = nc.dram_tensor("xt8", [NIN, NB], f8, kind="ExternalInput")
    # io[p, 2*s+o] = row index of operand o for neuron (slot s, partition p)
    io = nc.dram_tensor("io", [128, SLOTS * 2], i32, kind="ExternalInput")
    # ac[p, c, s] = coefficient A_c for neuron (slot s, partition p)
    ac = nc.dram_tensor("ac", [128, 4, SLOTS], f32, kind="ExternalInput")
    yt = nc.dram_tensor("yt", [NN, NB], f16, kind="ExternalOutput")

    with TileContext(nc) as tc:
        with tc.tile_pool(name="all", bufs=1) as pool:
            it = pool.tile([128, SLOTS * 2], i32)
            nc.sync.dma_start(it[:], io[:])
            act = pool.tile([128, 4, SLOTS], f32)
            nc.sync.dma_start(act[:], ac[:])

            # one small tile per gather: the SWDGE descriptor generator
            # mangles dest offsets beyond ~32KB within a single dest AP/tile,
            # so each indirect call targets its own tile at offset ~0.
            # ACT-path slots read g1 in fp8; the rest in f16.
            g1t = [
                pool.tile([128, NB], f8 if s < ACT_SLOTS else f16,
                          name=f"g1_{s}")
                for s in range(SLOTS)
            ]
            g2t = [pool.tile([128, NB], f16, name=f"g2_{s}") for s in range(SLOTS)]
            uv = [pool.tile([128, 2, NB], f16, name=f"uv{s}") for s in range(SLOTS)]
            ot = pool.tile([128, SLOTS, NB], f16)

            # Gather order: all g1 first in slot order (the serial ACT chain
            # and the DVE tensor_scalar block consume them gapless), then g2
            # rotated to match the TT chain's consumption order (TS slots
            # first). Completion-sem latency is ~3us, so bulk phases beat
            # finely interleaved just-in-time schedules here.
            order = [(s, 0) for s in range(SLOTS)] + \
                    [(s, 1) for s in list(range(ACT_SLOTS, SLOTS)) +
                     list(range(ACT_SLOTS))]
            for s, o in order:
                dst = g1t[s] if o == 0 else g2t[s]
                src = xt8 if (o == 0 and s < ACT_SLOTS) else xt
                nc.gpsimd.indirect_dma_start(
                    out=dst[:], out_offset=None,
                    in_=src[:],
                    in_offset=bass.IndirectOffsetOnAxis(
                        ap=it[:, 2 * s + o:2 * s + o + 1], axis=0),
                )

            # out = (Ap*g1 + A2)*g2 + (A1*g1 + A0):
            #   u = Ap*g1 + A2 ; v = A1*g1 + A0 ; out = u*g2 + v
            # Affines read only g1 (gathered first). Slots < ACT_SLOTS run
            # them on the scalar (ACT) engine; later slots run them as
            # 4x-packed DVE tensor_scalar ops, emitted FIRST so the in-order
            # DVE stream executes them during the g1-only phase, before the
            # g2-paced tensor_tensor chain.
            def slot_aps(s):
                return (g1t[s][:], g2t[s][:],
                        uv[s][:, 0, :], uv[s][:, 1, :],
                        act[:, 0, s:s + 1], act[:, 1, s:s + 1],
                        act[:, 2, s:s + 1], act[:, 3, s:s + 1])

            for s in range(ACT_SLOTS, SLOTS):
                g1, g2, u, v, A0, A1, A2, Ap = slot_aps(s)
                nc.vector.tensor_scalar(u, g1, Ap, A2,
                                        mybir.AluOpType.mult,
                                        mybir.AluOpType.add)
                nc.vector.tensor_scalar(v, g1, A1, A0,
                                        mybir.AluOpType.mult,
                                        mybir.AluOpType.add)

            # ACT engine stream: the serial chain u0,v0..u4,v4 paces the
            # ACT-path slots
            for s in range(ACT_SLOTS):
                g1, g2, u, v, A0, A1, A2, Ap = slot_aps(s)
                nc.scalar.activation(
                    u, g1, mybir.ActivationFunctionType.Identity,
                    bias=A2, scale=Ap)
                nc.scalar.activation(
                    v, g1, mybir.ActivationFunctionType.Identity,
                    bias=A0, scale=A1)

            # DVE tensor_tensor chain + writes: TS slots first (their u,v and
            # g2 are ready earliest), then the ACT slots in order
            tt_order = list(range(ACT_SLOTS, SLOTS)) + list(range(ACT_SLOTS))
            for s in tt_order:
                g1, g2, u, v, A0, A1, A2, Ap = slot_aps(s)
                nc.vector.tensor_mul(ot[:, s, :], u, g2)
                nc.vector.tensor_add(ot[:, s, :], ot[:, s, :], v)
                if s == tt_order[-1]:
                    # final write split across two HWDGE queues so its data
                    # and completion receipt land sooner
                    h = NB // 2
                    nc.sync.dma_start(yt[s * 128:(s + 1) * 128, 0:h],
                                      ot[:, s, 0:h])
                    nc.scalar.dma_start(yt[s * 128:(s + 1) * 128, h:NB],
                                        ot[:, s, h:NB])
                else:
                    nc.sync.dma_start(yt[s * 128:(s + 1) * 128, :], ot[:, s, :])

    nc.compile()
    return nc


def _prep_core_inputs(x, w, conn_indices):
    """Host-side shard/layout prep. Returns list of per-core input dicts."""
    import ml_dtypes

    xT = x.T
    xt = np.ascontiguousarray(xT.astype(np.float16))  # [NIN, BATCH] f16, shared
    xt8 = np.ascontiguousarray(
        xT.astype(ml_dtypes.float8_e3m4)).view(np.uint8)
    # A = softmax(w) @ GATE_COEF, [NNEUR, 4] — tiny; compute on host in f64
    ew = np.exp(w.astype(np.float64))
    probs = ew / ew.sum(axis=1, keepdims=True)
    A = (probs @ GATE_COEF.astype(np.float64)).astype(np.float32)
    maps = []
    for c in range(NCORES):
        n0 = c * NN
        # neuron n0 + s*128 + p -> partition p, slot s
        idx = conn_indices[n0:n0 + NN, :].reshape(SLOTS, 128, 2)
        io = idx.transpose(1, 0, 2).reshape(128, SLOTS * 2)
        ac = A[n0:n0 + NN, :].reshape(SLOTS, 128, 4).transpose(1, 2, 0)
        maps.append({
            "xt": xt,
            "xt8": xt8,
            "io": np.ascontiguousarray(io).astype(np.int32),
            "ac": np.ascontiguousarray(ac),
        })
    return maps


def run_cores(in_maps, trace=False):
    from concourse.bass_utils import run_bass_kernel_spmd

    if "nc" not in _CACHE:
        _CACHE["nc"] = _build_nc()
    return run_bass_kernel_spmd(
        _CACHE["nc"], in_maps, core_ids=list(range(NCORES)), trace=trace
    )


def _assemble(results):
    out = np.empty((BATCH, NNEUR), dtype=np.float32)
    for c in range(NCORES):
        n0 = c * NN
        out[:, n0:n0 + NN] = results[c]["yt"].T.astype(np.float32)
    return out


def kernel(x, w, conn_indices):
    x = np.asarray(x, dtype=np.float32)
    w = np.asarray(w, dtype=np.float32)
    conn_indices = np.asarray(conn_indices)
    in_maps = _prep_core_inputs(x, w, conn_indices)
    res = run_cores(in_maps)
    return _assemble([r for r in res.results])


# revision 31
# speedup vs baseline: 1.0699x; 1.0139x over previous
"""LogicLayer Trainium2 kernel.

out[b, n] = sum_k softmax(w[n])_k * gate_k(a1, a2),  a1 = x[b, i1[n]], a2 = x[b, i2[n]]

All 16 differentiable gates are affine in {1, a1, a2, a1*a2}:
    out[b, n] = A0[n] + A1[n]*a1 + A2[n]*a2 + Ap[n]*a1*a2
with A* = softmax(w[n]) @ C for a constant [16, 4] table C. A* is tiny
([8192, 4] total) and is precomputed on the host, so the device kernel is a
pure gather + 2-DVE-ops-per-slot + write pipeline.

Device plan (8 NeuronCores, neuron-sharded: 1024 neurons x full 2048 batch each):
  - x is shipped transposed and cast to f16 (xt [8192, 2048]) so one neuron's
    input column is a contiguous 4KB row in HBM. The correctness gate is
    rel_err < 2e-2; f16 gathers + f16 output writes land at ~3e-4 while
    halving HBM traffic vs f32 (~12MB/core instead of ~24MB).
  - gathers: 16 single-offset gpsimd.indirect_dma_start calls (multi-offset
    offset-APs and >32KB dest offsets within one dest tile both break the
    real SWDGE descriptor generator, although CoreSim accepts them), each
    into its own small per-slot tile.
  - all working tiles are static f16 SBUF residents (~100KB/partition), so
    there is no pool-buffer recycling and no resulting Q7 stalls.
  - inner loop is 2 fused DVE ops per 128-neuron slot (f16 data hits the
    2x-packed DVE mode):
        t   = (Ap*g2 + A1) * g1          (affine_mul_reduce)
        out = (A2*g2 + A0) + t           (affine_then_add)
  - output is written neuron-major [1024, 2048] f16, one write per slot so
    writes overlap the remaining gathers; host reassembles/transposes/upcasts.
"""

import numpy as np

BATCH = 2048
NIN = 8192
NNEUR = 8192
NCORES = 8
NN = NNEUR // NCORES  # neurons per core (1024)
NB = BATCH            # full batch per core
SLOTS = NN // 128     # 8
ACT_SLOTS = 4         # slots < this offload their affines to the ACT engine;
                      # later slots use DVE tensor_scalar so the tail after the
                      # last gather avoids the cross-engine ACT chain


# gate -> (c0, c1, c2, cp) so gate_k(a1,a2) = c0 + c1*a1 + c2*a2 + cp*a1*a2
GATE_COEF = np.array(
    [
        [0, 0, 0, 0],    # FALSE
        [0, 0, 0, 1],    # AND
        [0, 1, 0, -1],   # a1 AND NOT a2
        [0, 1, 0, 0],    # a1
        [0, 0, 1, -1],   # NOT a1 AND a2
        [0, 0, 1, 0],    # a2
        [0, 1, 1, -2],   # XOR
        [0, 1, 1, -1],   # OR
        [1, -1, -1, 1],  # NOR
        [1, -1, -1, 2],  # XNOR
        [1, 0, -1, 0],   # NOT a2
        [1, 0, -1, 1],   # a1 OR NOT a2
        [1, -1, 0, 0],   # NOT a1
        [1, -1, 0, 1],   # NOT a1 OR a2
        [1, 0, 0, -1],   # NAND
        [1, 0, 0, 0],    # TRUE
    ],
    dtype=np.float32,
)  # [16, 4]

_CACHE = {}


def _build_nc():
    import concourse.bacc as bacc
    import concourse.bass as bass
    import concourse.mybir as mybir
    from concourse.tile import TileContext

    f32 = mybir.dt.float32
    f16 = mybir.dt.float16
    f8 = mybir.dt.float8e3
    i32 = mybir.dt.int32

    nc = bacc.Bacc("TRN2")
    xt = nc.dram_tensor("xt", [NIN, NB], f16, kind="ExternalInput")
    # fp8-e3m4 copy of xt: the ACT-slot g1 gathers read this (the ACT engine
    # upconverts on read), saving HBM bytes at ~1.8e-3 total rel err
    xt8 = nc.dram_tensor("xt8", [NIN, NB], f8, kind="ExternalInput")
    # io[p, 2*s+o] = row index of operand o for neuron (slot s, partition p)
    io = nc.dram_tensor("io", [128, SLOTS * 2], i32, kind="ExternalInput")
    # ac[p, c, s] = coefficient A_c for neuron (slot s, partition p)
    ac = nc.dram_tensor("ac", [128, 4, SLOTS], f32, kind="ExternalInput")
    yt = nc.dram_tensor("yt", [NN, NB], f16, kind="ExternalOutput")

    with TileContext(nc) as tc:
        with tc.tile_pool(name="all", bufs=1) as pool:
            it = pool.tile([128, SLOTS * 2], i32)
            nc.sync.dma_start(it[:], io[:])
            act = pool.tile([128, 4, SLOTS], f32)
            nc.sync.dma_start(act[:], ac[:])

            # one small tile per gather: the SWDGE descriptor generator
            # mangles dest offsets beyond ~32KB within a single dest AP/tile,
            # so each indirect call targets its own tile at offset ~0.
            # ACT-path slots read g1 in fp8; the rest in f16.
            g1t = [
                pool.tile([128, NB], f8 if s < ACT_SLOTS else f16,
                          name=f"g1_{s}")
                for s in range(SLOTS)
            ]
            g2t = [pool.tile([128, NB], f16, name=f"g2_{s}") for s in range(SLOTS)]
            uv = [pool.tile([128, 2, NB], f16, name=f"uv{s}") for s in range(SLOTS)]
            ot = pool.tile([128, SLOTS, NB], f16)

            # Gather order: slot 0's g1 first (starts the serial ACT chain),
            # then the TS slots' g1+g2 pairs back-to-back so the DVE's
            # interleaved TS/TT stream starts ~8us earlier, then the
            # remaining ACT-slot tensors in consumption order.
            order = [(0, 0)]
            for s in range(ACT_SLOTS, SLOTS):
                order += [(s, 0), (s, 1)]
            order += [(1, 0), (0, 1), (2, 0), (1, 1), (3, 0), (2, 1), (3, 1)]
            assert len(order) == 2 * SLOTS and len(set(order)) == 2 * SLOTS
            for s, o in order:
                dst = g1t[s] if o == 0 else g2t[s]
                src = xt8 if (o == 0 and s < ACT_SLOTS) else xt
                nc.gpsimd.indirect_dma_start(
                    out=dst[:], out_offset=None,
                    in_=src[:],
                    in_offset=bass.IndirectOffsetOnAxis(
                        ap=it[:, 2 * s + o:2 * s + o + 1], axis=0),
                )

            # out = (Ap*g1 + A2)*g2 + (A1*g1 + A0):
            #   u = Ap*g1 + A2 ; v = A1*g1 + A0 ; out = u*g2 + v
            # Affines read only g1 (gathered first). Slots < ACT_SLOTS run
            # them on the scalar (ACT) engine; later slots run them as
            # 4x-packed DVE tensor_scalar ops, emitted FIRST so the in-order
            # DVE stream executes them during the g1-only phase, before the
            # g2-paced tensor_tensor chain.
            def slot_aps(s):
                return (g1t[s][:], g2t[s][:],
                        uv[s][:, 0, :], uv[s][:, 1, :],
                        act[:, 0, s:s + 1], act[:, 1, s:s + 1],
                        act[:, 2, s:s + 1], act[:, 3, s:s + 1])

            # ACT engine stream: the serial chain u0,v0..u3,v3 paces the
            # ACT-path slots
            for s in range(ACT_SLOTS):
                g1, g2, u, v, A0, A1, A2, Ap = slot_aps(s)
                nc.scalar.activation(
                    u, g1, mybir.ActivationFunctionType.Identity,
                    bias=A2, scale=Ap)
                nc.scalar.activation(
                    v, g1, mybir.ActivationFunctionType.Identity,
                    bias=A0, scale=A1)

            # DVE stream + writes: TS slots first with their tensor_scalar
            # affines interleaved right before their TTs (each slot's inputs
            # arrive as consecutive gather calls), then the ACT slots
            tt_order = list(range(ACT_SLOTS, SLOTS)) + list(range(ACT_SLOTS))
            for s in tt_order:
                g1, g2, u, v, A0, A1, A2, Ap = slot_aps(s)
                if s >= ACT_SLOTS:
                    nc.vector.tensor_scalar(u, g1, Ap, A2,
                                            mybir.AluOpType.mult,
                                            mybir.AluOpType.add)
                    nc.vector.tensor_scalar(v, g1, A1, A0,
                                            mybir.AluOpType.mult,
                                            mybir.AluOpType.add)
                nc.vector.tensor_mul(ot[:, s, :], u, g2)
                nc.vector.tensor_add(ot[:, s, :], ot[:, s, :], v)
                if s == tt_order[-1]:
                    # final write split across two HWDGE queues so its data
                    # and completion receipt land sooner
                    h = NB // 2
                    nc.sync.dma_start(yt[s * 128:(s + 1) * 128, 0:h],
                                      ot[:, s, 0:h])
                    nc.scalar.dma_start(yt[s * 128:(s + 1) * 128, h:NB],
                                        ot[:, s, h:NB])
                else:
                    nc.sync.dma_start(yt[s * 128:(s + 1) * 128, :], ot[:, s, :])

    nc.compile()
    return nc


def _prep_core_inputs(x, w, conn_indices):
    """Host-side shard/layout prep. Returns list of per-core input dicts."""
    import ml_dtypes

    xT = x.T
    xt = np.ascontiguousarray(xT.astype(np.float16))  # [NIN, BATCH] f16, shared
    xt8 = np.ascontiguousarray(
        xT.astype(ml_dtypes.float8_e3m4)).view(np.uint8)
    # A = softmax(w) @ GATE_COEF, [NNEUR, 4] — tiny; compute on host in f64
    ew = np.exp(w.astype(np.float64))
    probs = ew / ew.sum(axis=1, keepdims=True)
    A = (probs @ GATE_COEF.astype(np.float64)).astype(np.float32)
    maps = []
    for c in range(NCORES):
        n0 = c * NN
        # neuron n0 + s*128 + p -> partition p, slot s
        idx = conn_indices[n0:n0 + NN, :].reshape(SLOTS, 128, 2)
        io = idx.transpose(1, 0, 2).reshape(128, SLOTS * 2)
        ac = A[n0:n0 + NN, :].reshape(SLOTS, 128, 4).transpose(1, 2, 0)
        maps.append({
            "xt": xt,
            "xt8": xt8,
            "io": np.ascontiguousarray(io).astype(np.int32),
            "ac": np.ascontiguousarray(ac),
        })
    return maps


def run_cores(in_maps, trace=False):
    from concourse.bass_utils import run_bass_kernel_spmd

    if "nc" not in _CACHE:
        _CACHE["nc"] = _build_nc()
    return run_bass_kernel_spmd(
        _CACHE["nc"], in_maps, core_ids=list(range(NCORES)), trace=trace
    )


def _assemble(results):
    out = np.empty((BATCH, NNEUR), dtype=np.float32)
    for c in range(NCORES):
        n0 = c * NN
        out[:, n0:n0 + NN] = results[c]["yt"].T.astype(np.float32)
    return out


def kernel(x, w, conn_indices):
    x = np.asarray(x, dtype=np.float32)
    w = np.asarray(w, dtype=np.float32)
    conn_indices = np.asarray(conn_indices)
    in_maps = _prep_core_inputs(x, w, conn_indices)
    res = run_cores(in_maps)
    return _assemble([r for r in res.results])
